# revision 2
# baseline (speedup 1.0000x reference)
"""Trainium2 Bass kernel for nn_ConditionalSoftmax (sampled-softmax NLL loss).

Computes, for each batch row b:
    v_c   = vectors[cs[b]]                      # [D]
    h     = relu(v_c @ W1 + b1)                 # [H]
    logit = h @ W2 + b2                         # [V]
    nll_b = logsumexp(logit) - logit[v2s[ws[b]]]

Sharding: data-parallel over batch across 8 NeuronCores (1024 rows/core),
weights replicated.  Per core the dominant work is the [1024,512]@[512,20000]
matmul; W2 is streamed through SBUF once in bf16 (full-rate on the PE vs 1/4
rate for fp32) in 500-column tiles, logits accumulate in PSUM and are reduced
in place by the ScalarEngine's fused exp+row-sum (accum_out) so the [1024,20000]
logit matrix never touches HBM.  The target logit takes a separate cheap path:
indirect-gather of the needed W2.T rows and a fused multiply-reduce on the
VectorEngine against an fp32 recompute of h.
"""

import numpy as np
import ml_dtypes

import concourse.bass as bass
import concourse.mybir as mybir
import concourse.tile as tile
from concourse import bacc, bass_utils
from concourse.bass import IndirectOffsetOnAxis, ts
from concourse.masks import make_identity

# Problem shapes (hardcoded per contest contract)
N_VOCAB = 50000
V = 20000
D = 300
DP = 384          # D padded to 3*128
NDC = 3           # contraction chunks for D
H = 512
NHC = 4           # contraction chunks for H
B = 8192
NCORES = 8
BL = B // NCORES  # 1024 rows per core
NBT = BL // 128   # 8 batch tiles of 128 rows
VT = 500          # vocab tile width (one PSUM bank = 500 fp32 cols + slack)
NVT = V // VT     # 40 vocab tiles

F32 = mybir.dt.float32
BF16 = mybir.dt.bfloat16
I32 = mybir.dt.int32
AF = mybir.ActivationFunctionType
OP = mybir.AluOpType

_BUILD_CACHE = {}


def _build(b1_nz: bool, b2_nz: bool):
    key = (b1_nz, b2_nz)
    if key in _BUILD_CACHE:
        return _BUILD_CACHE[key]

    nc = bacc.Bacc(
        "TRN2",
        target_bir_lowering=False,
        debug=False,
        num_devices=NCORES,
    )

    cs_idx = nc.dram_tensor("cs_idx", [NBT, 128, 1], I32, kind="ExternalInput").ap()
    ws_idx = nc.dram_tensor("ws_idx", [NBT, 128, 1], I32, kind="ExternalInput").ap()
    vectors = nc.dram_tensor("vectors", [N_VOCAB, D], F32, kind="ExternalInput").ap()
    v2s = nc.dram_tensor("v2s", [N_VOCAB, 1], I32, kind="ExternalInput").ap()
    w1 = nc.dram_tensor("w1", [DP, H], BF16, kind="ExternalInput").ap()
    b1c = nc.dram_tensor("b1c", [NHC, 128, 1], F32, kind="ExternalInput").ap()
    w2 = nc.dram_tensor("w2", [H, V], BF16, kind="ExternalInput").ap()
    w2tb = nc.dram_tensor("w2tb", [V, H + 1], F32, kind="ExternalInput").ap()
    if b1_nz:
        b1rep = nc.dram_tensor("b1rep", [128, H], F32, kind="ExternalInput").ap()
    if b2_nz:
        b2rep = nc.dram_tensor("b2rep", [128, V], F32, kind="ExternalInput").ap()
    nll = nc.dram_tensor("nll", [NBT, 128, 1], F32, kind="ExternalOutput").ap()

    with tile.TileContext(nc) as tc:
        with (
            tc.tile_pool(name="consts", bufs=1) as consts,
            tc.tile_pool(name="idx", bufs=4) as idxp,
            tc.tile_pool(name="vc", bufs=4) as vcp,
            tc.tile_pool(name="gw", bufs=4) as gwp,
            tc.tile_pool(name="w2t", bufs=3) as w2p,
            tc.tile_pool(name="scr", bufs=2) as scrp,
            tc.tile_pool(name="ps1", bufs=3, space="PSUM") as ps1,
            tc.tile_pool(name="psmain", bufs=4, space="PSUM") as psm,
        ):
            ident = consts.tile([128, 128], BF16)
            make_identity(nc, ident[:])

            w1sb = consts.tile([128, NDC, H], BF16)
            nc.sync.dma_start(w1sb[:], w1.rearrange("(c p) h -> p c h", p=128))
            b1sb = consts.tile([128, NHC], F32)
            for hc in range(NHC):
                nc.sync.dma_start(b1sb[:, hc : hc + 1], b1c[hc])
            if b1_nz:
                b1rep_sb = consts.tile([128, H], F32)
                nc.sync.dma_start(b1rep_sb[:], b1rep[:])

            # Long-lived activations
            vcT = consts.tile([128, NDC, BL], BF16)    # v_c^T, d-major
            hT = consts.tile([128, NHC, BL], BF16)     # h^T, h-major (PE input)
            hb = consts.tile([128, NBT, H], F32)       # h, batch-major (target dot)
            sums = consts.tile([128, NBT * NVT], F32)  # per-(b,v) exp partial sums
            tdot = consts.tile([128, NBT], F32)        # target logits
            fin = consts.tile([128, 3 * NBT], F32)     # S | lnS | result

            # ---- Phase 1: gather embeddings, transpose, first layer ----
            for t in range(NBT):
                cidx = idxp.tile([128, 1], I32, tag="cidx")
                nc.sync.dma_start(cidx[:], cs_idx[t])
                vc = vcp.tile([128, D], F32, tag="vc")
                nc.gpsimd.indirect_dma_start(
                    out=vc[:],
                    out_offset=None,
                    in_=vectors[:],
                    in_offset=IndirectOffsetOnAxis(ap=cidx[:, :1], axis=0),
                )
                vcb = vcp.tile([128, DP], BF16, tag="vcb")
                nc.vector.memset(vcb[:, D:DP], 0.0)
                nc.vector.tensor_copy(vcb[:, :D], vc[:])
                for c in range(NDC):
                    pt = ps1.tile([128, 128], BF16, tag="ps1")
                    nc.tensor.transpose(pt[:], vcb[:, ts(c, 128)], ident[:])
                    nc.vector.tensor_copy(vcT[:, c, ts(t, 128)], pt[:])

                # h^T tiles for this batch tile: [128h x 128b] per h-chunk
                for hc in range(NHC):
                    ph = ps1.tile([128, 128], F32, tag="ps1")
                    for c in range(NDC):
                        nc.tensor.matmul(
                            ph[:],
                            lhsT=w1sb[:, c, ts(hc, 128)],
                            rhs=vcT[:, c, ts(t, 128)],
                            start=(c == 0),
                            stop=(c == NDC - 1),
                        )
                    nc.scalar.activation(
                        hT[:, hc, ts(t, 128)], ph[:], AF.Relu,
                        bias=b1sb[:, hc : hc + 1], scale=1.0,
                    )

                # batch-major h (fp32) for the target-logit dot
                phb = ps1.tile([128, H], F32, tag="ps1")
                for c in range(NDC):
                    nc.tensor.matmul(
                        phb[:],
                        lhsT=vcT[:, c, ts(t, 128)],
                        rhs=w1sb[:, c, :],
                        start=(c == 0),
                        stop=(c == NDC - 1),
                    )
                if b1_nz:
                    nc.vector.tensor_add(phb[:], phb[:], b1rep_sb[:])
                nc.scalar.activation(hb[:, t, :], phb[:], AF.Relu, bias=0.0, scale=1.0)

                # target support index + gathered W2.T row (with b2 in col H)
                widx = idxp.tile([128, 1], I32, tag="widx")
                nc.sync.dma_start(widx[:], ws_idx[t])
                sidx = idxp.tile([128, 1], I32, tag="sidx")
                nc.gpsimd.indirect_dma_start(
                    out=sidx[:],
                    out_offset=None,
                    in_=v2s[:],
                    in_offset=IndirectOffsetOnAxis(ap=widx[:, :1], axis=0),
                )
                g = gwp.tile([128, H + 1], F32, tag="g")
                nc.gpsimd.indirect_dma_start(
                    out=g[:],
                    out_offset=None,
                    in_=w2tb[:],
                    in_offset=IndirectOffsetOnAxis(ap=sidx[:, :1], axis=0),
                )
                # (tensor_tensor_reduce is broken on this HW path; use 3 ops)
                gscr = gwp.tile([128, H], F32, tag="gscr")
                nc.vector.tensor_mul(gscr[:], hb[:, t, :], g[:, :H])
                gacc = gwp.tile([128, 1], F32, tag="gacc")
                nc.vector.reduce_sum(
                    out=gacc[:], in_=gscr[:], axis=mybir.AxisListType.X
                )
                nc.vector.tensor_add(tdot[:, t : t + 1], gacc[:], g[:, H : H + 1])

            # ---- Phase 2: stream W2, logits in PSUM, fused exp+rowsum ----
            w2r = w2.rearrange("(k p) v -> p k v", p=128)
            for v in range(NVT):
                w2t = w2p.tile([128, NHC, VT], BF16, tag="w2t")
                nc.sync.dma_start(w2t[:], w2r[:, :, ts(v, VT)])
                if b2_nz:
                    b2t = w2p.tile([128, VT], F32, tag="b2t")
                    nc.sync.dma_start(b2t[:], b2rep[:, ts(v, VT)])
                for t in range(NBT):
                    ps = psm.tile([128, VT], F32, tag="ps")
                    for k in range(NHC):
                        nc.tensor.matmul(
                            ps[:],
                            lhsT=hT[:, k, ts(t, 128)],
                            rhs=w2t[:, k, :],
                            start=(k == 0),
                            stop=(k == NHC - 1),
                        )
                    if b2_nz:
                        nc.vector.tensor_add(ps[:], ps[:], b2t[:])
                    escr = scrp.tile([128, VT], F32, tag="escr")
                    nc.scalar.activation(
                        escr[:], ps[:], AF.Exp,
                        accum_out=sums[:, t * NVT + v : t * NVT + v + 1],
                    )

            # ---- Phase 3: logsumexp and output ----
            for t in range(NBT):
                S = fin[:, t : t + 1]
                nc.vector.reduce_sum(
                    out=S, in_=sums[:, ts(t, NVT)], axis=mybir.AxisListType.X
                )
                lnS = fin[:, NBT + t : NBT + t + 1]
                nc.scalar.activation(lnS, S, AF.Ln)
                res = fin[:, 2 * NBT + t : 2 * NBT + t + 1]
                nc.vector.tensor_sub(res, lnS, tdot[:, t : t + 1])
                nc.sync.dma_start(nll[t], res)

    nc.compile()
    _BUILD_CACHE[key] = nc
    return nc


def _prep_inputs(ws, cs, vectors, W1, b1, W2, b2, vector_to_support):
    ws = np.asarray(ws)
    cs = np.asarray(cs)
    vectors = np.asarray(vectors, dtype=np.float32)
    W1 = np.asarray(W1, dtype=np.float32)
    b1 = np.asarray(b1, dtype=np.float32)
    W2 = np.asarray(W2, dtype=np.float32)
    b2 = np.asarray(b2, dtype=np.float32)
    v2s = np.asarray(vector_to_support)

    b1_nz = bool(np.any(b1))
    b2_nz = bool(np.any(b2))

    w1p = np.zeros((DP, H), dtype=ml_dtypes.bfloat16)
    w1p[:D] = W1.astype(ml_dtypes.bfloat16)
    w2bf = np.ascontiguousarray(W2.astype(ml_dtypes.bfloat16))
    w2tb = np.ascontiguousarray(
        np.concatenate([W2.T, b2[:, None]], axis=1).astype(np.float32)
    )
    b1c = np.ascontiguousarray(b1.reshape(NHC, 128, 1))
    v2s2d = np.ascontiguousarray(v2s.astype(np.int32).reshape(N_VOCAB, 1))

    shared = {
        "vectors": np.ascontiguousarray(vectors),
        "v2s": v2s2d,
        "w1": w1p,
        "b1c": b1c,
        "w2": w2bf,
        "w2tb": w2tb,
    }
    if b1_nz:
        shared["b1rep"] = np.ascontiguousarray(
            np.broadcast_to(b1, (128, H)).astype(np.float32)
        )
    if b2_nz:
        shared["b2rep"] = np.ascontiguousarray(
            np.broadcast_to(b2, (128, V)).astype(np.float32)
        )

    in_maps = []
    for c in range(NCORES):
        sl = slice(c * BL, (c + 1) * BL)
        m = dict(shared)
        m["cs_idx"] = np.ascontiguousarray(
            cs[sl].astype(np.int32).reshape(NBT, 128, 1)
        )
        m["ws_idx"] = np.ascontiguousarray(
            ws[sl].astype(np.int32).reshape(NBT, 128, 1)
        )
        in_maps.append(m)
    return in_maps, b1_nz, b2_nz


def run(inputs: dict, trace: bool = False):
    """Run the SPMD kernel. Returns (output [B] fp32, BassKernelResults)."""
    in_maps, b1_nz, b2_nz = _prep_inputs(**inputs)
    nc = _build(b1_nz, b2_nz)
    res = bass_utils.run_bass_kernel_spmd(
        nc, in_maps, core_ids=list(range(NCORES)), trace=trace
    )
    out = np.concatenate(
        [r["nll"].reshape(-1) for r in res.results]
    ).astype(np.float32)
    return out, res


def kernel(**inputs) -> np.ndarray:
    out, _ = run(inputs, trace=False)
    return out
